# revision 21
# baseline (speedup 1.0000x reference)
"""Trainium2 Bass kernel for nn_DglHGTGRUConv (HGT conv + GRU, T=2 steps). v2

Strategy (8 NeuronCores, SPMD):
  - Nodes range-sharded; edges sharded by DST range => segment softmax and
    scatter-sum are core-local.
  - bf16 KV table: per step each core projects its h-shard to K|V (bf16),
    one AllGather builds the full KV table (bf16, half the bytes of v1).
  - Edge phase per relation, groups of 4 chunks (128 edges, <=32 slots each):
      * 4 indirect-DMA row gathers of KV[src] per group (bf16 rows, 512B)
      * batched dma_gather (int16 idx) of q rows per slot across NB groups
      * one-hot built on device (iota + is_equal), transposed on PE
      * SDDMM + exp + one-hot segment-sum matmuls (all bf16 operands)
      * per-relation BDm transform + normalize
      * batched dma_scatter_add (int16 idx) into one shared pre-zeroed agg
  - h kept SBUF-resident (f32 rows + bf16 transposed copies) for both steps;
    GRU's hidden-side matmul recomputed locally (no gh DRAM roundtrip).

All graph structure is computed on the host from src/dst; the device program
is identical on all cores, per-core structure arrives as input data.
"""
import os
import sys
sys.path.insert(0, "/opt/trn_rl_repo")
import numpy as np

DBG_RELS = int(os.environ.get("KB_RELS", "3"))
DBG_STEPS = int(os.environ.get("KB_STEPS", "2"))
DBG_GCAP = int(os.environ.get("KB_GCAP", "100000"))
DBG_ELVL = int(os.environ.get("KB_ELVL", "9"))
DBG_E2 = int(os.environ.get("KB_E2", "9"))
DBG_C4MAX = int(os.environ.get("KB_C4MAX", "4"))

import concourse.bass as bass
import concourse.bacc as bacc
import concourse.tile as tile
import concourse.mybir as mybir
from concourse.masks import make_identity

f32 = mybir.dt.float32
bf16 = mybir.dt.bfloat16
i32 = mybir.dt.int32
i16 = mybir.dt.int16
AF = mybir.ActivationFunctionType
ALU = mybir.AluOpType
AX = mybir.AxisListType

N = 100000
IN = 128
OUT = 128
H = 8
DK = OUT // H
R = 3
T = 2
NCORES = 8
SH = N // NCORES          # 12500 real rows per shard
SHP = 12544               # padded shard rows (98 * 128)
NTILE = SHP // 128        # 98
KVN = SHP * NCORES        # 100352 rows in the allgathered KV table
AGGR = 13312              # agg rows (12544 real + garbage pad region)
GARB = SHP                # garbage row base for pad slots
EDGE_CAP = 128
SLOT_CAP = 32
NB = int(os.environ.get("KB_NB", "7"))  # groups per batch (<=7: a dma_gather/scatter batch must fit the 1024-descriptor SWDGE ring)
ST = 2                    # dense/GRU supertile (tiles per iteration)


# --------------------------------------------------------------------------
# Host-side graph packing
# --------------------------------------------------------------------------

def _pack_core_relation(src_g, dst_l):
    """Pack one core's edges of one relation (dst-local ids, any order)."""
    order = np.argsort(dst_l, kind="stable")
    src_g = src_g[order]
    dst_l = dst_l[order]
    chunks = []
    cur_edges, cur_slots = [], []
    i = 0
    M = len(dst_l)
    while i < M:
        j = i
        d = dst_l[i]
        while j < M and dst_l[j] == d:
            j += 1
        seg = j - i
        if seg > EDGE_CAP:
            raise ValueError(f"segment larger than {EDGE_CAP} not supported")
        if len(cur_edges) + seg > EDGE_CAP or len(cur_slots) + 1 > SLOT_CAP:
            chunks.append((cur_edges, cur_slots))
            cur_edges, cur_slots = [], []
        s = len(cur_slots)
        cur_slots.append(int(d))
        for e in range(i, j):
            cur_edges.append((int(src_g[e]), s))
        i = j
    if cur_edges:
        chunks.append((cur_edges, cur_slots))
    return chunks


def _remap(n):
    """global node id -> padded KV-table row"""
    return (n // SH) * SHP + (n % SH)


def _wrap16(arr):
    """int16 list -> [128, ceil(n/16)] wrapped in 16 partitions, replicated
    across the 8 16-partition groups (dma_gather/scatter idx layout)."""
    n = len(arr)
    cols = (n + 15) // 16
    out = np.zeros((128, cols), np.int16)
    for j, v in enumerate(arr):
        for rep in range(8):
            out[rep * 16 + (j % 16), j // 16] = v
    return out


def build_graph_data(src, dst):
    src = np.asarray(src).astype(np.int64)
    dst = np.asarray(dst).astype(np.int64)
    packed = [[None] * R for _ in range(NCORES)]
    nch = 0
    for r in range(R):
        for c in range(NCORES):
            lo, hi = c * SH, (c + 1) * SH
            sel = (dst[r] >= lo) & (dst[r] < hi)
            chunks = _pack_core_relation(src[r][sel], dst[r][sel] - lo)
            packed[c][r] = chunks
            nch = max(nch, len(chunks))
    G = (nch + 3) // 4
    NCHUNK = G * 4

    per_core = []
    for c in range(NCORES):
        meta = np.zeros((R, G * 128, 4), np.int32)
        mslot = np.zeros((R, G * 128, 4), np.int16)
        qidx = np.zeros((R, 128, G * 8), np.int16)
        sidx = np.zeros((R, 128, G * 8), np.int16)
        for r in range(R):
            chunks = packed[c][r]
            srcidx = np.zeros((NCHUNK, 128), np.int32)
            slotv = np.full((NCHUNK, 128), -1, np.int32)
            qlist = np.zeros(G * 128, np.int16)
            slist = np.zeros(G * 128, np.int16)
            for j in range(G * 128):
                slist[j] = GARB + (j & 63)
            for ci in range(NCHUNK):
                edges, slots = ([], []) if ci >= len(chunks) else chunks[ci]
                c4 = ci % 4
                g = ci // 4
                for p, (s_glob, s) in enumerate(edges):
                    srcidx[ci, p] = _remap(s_glob)
                    slotv[ci, p] = s + 32 * (c4 & 1)
                for s, d in enumerate(slots):
                    j = g * 128 + c4 * 32 + s
                    qlist[j] = d
                    slist[j] = d
            m = meta[r].reshape(G, 128, 4)
            ms = mslot[r].reshape(G, 128, 4)
            for g in range(G):
                for c4 in range(4):
                    ci = g * 4 + c4
                    m[g, :, c4] = srcidx[ci]
                    ms[g, :, c4] = slotv[ci]
            qidx[r] = _wrap16(qlist)
            sidx[r] = _wrap16(slist)
        per_core.append({
            "meta": meta.reshape(R * G * 128, 4),
            "mslot": mslot.reshape(R * G * 128, 4),
            "qidx": qidx.reshape(R * 128, G * 8),
            "sidx": sidx.reshape(R * 128, G * 8),
        })
    return G, per_core


# --------------------------------------------------------------------------
# Host-side weight folding
# --------------------------------------------------------------------------

def _bf(x):
    import ml_dtypes
    return x.astype(np.float32).astype(ml_dtypes.bfloat16)


def fold_weights(inp):
    d64 = {k: np.asarray(v).astype(np.float64) for k, v in inp.items()
           if k not in ("h", "src", "dst")}
    Wk, bk = d64["Wk"], d64["bk"]
    Wq, bq = d64["Wq"], d64["bq"]
    Wv, bv = d64["Wv"], d64["bv"]
    Wa, ba = d64["Wa"], d64["ba"]
    rel_pri, rel_att, rel_msg = d64["rel_pri"], d64["rel_att"], d64["rel_msg"]
    W_ih, b_ih = d64["W_ih"], d64["b_ih"]
    W_hh, b_hh = d64["W_hh"], d64["b_hh"]

    Wkv = np.concatenate([Wk.T, Wv.T], axis=1)                 # [128, 256]
    bkv = np.concatenate([bk, bv])                             # [256]

    sqrt_dk = np.sqrt(DK)
    Wqr = np.zeros((IN, R * OUT))
    bqr = np.zeros((R * OUT,))
    BDm = np.zeros((R, OUT, OUT))
    for r in range(R):
        BDq = np.zeros((OUT, OUT))
        for h in range(H):
            BDq[h * DK:(h + 1) * DK, h * DK:(h + 1) * DK] = \
                rel_att[r, h].T * (rel_pri[r, h] / sqrt_dk)
            BDm[r, h * DK:(h + 1) * DK, h * DK:(h + 1) * DK] = rel_msg[r, h]
        Wqr[:, r * OUT:(r + 1) * OUT] = Wq.T @ BDq
        bqr[r * OUT:(r + 1) * OUT] = bq @ BDq

    ones = np.ones((128, 1))
    f = np.float32
    out = {
        "Wkv": _bf(Wkv), "bkv_x": (ones * bkv[None, :]).astype(f),
        "Wqr": _bf(Wqr), "bqr_x": (ones * bqr[None, :]).astype(f),
        "BDm": _bf(np.concatenate([BDm[r] for r in range(R)], axis=1)),
        "WaT": _bf(Wa.T), "ba_x": (ones * ba[None, :]).astype(f),
        "WihT": _bf(W_ih.T), "bih_x": (ones * b_ih[None, :]).astype(f),
        "WhhT": _bf(W_hh.T), "bhh_x": (ones * b_hh[None, :]).astype(f),
    }
    return out


# --------------------------------------------------------------------------
# Device program
# --------------------------------------------------------------------------

def build_program(G, nsteps=None):
    if nsteps is None:
        nsteps = DBG_STEPS
    nc = bacc.Bacc("TRN2", target_bir_lowering=False, debug=False,
                   num_devices=NCORES)

    h_in = nc.dram_tensor("h_in", [SHP, IN], f32, kind="ExternalInput")
    Wkv = nc.dram_tensor("Wkv", [IN, 2 * OUT], bf16, kind="ExternalInput")
    bkv_x = nc.dram_tensor("bkv_x", [128, 2 * OUT], f32, kind="ExternalInput")
    Wqr = nc.dram_tensor("Wqr", [IN, R * OUT], bf16, kind="ExternalInput")
    bqr_x = nc.dram_tensor("bqr_x", [128, R * OUT], f32, kind="ExternalInput")
    BDm = nc.dram_tensor("BDm", [OUT, R * OUT], bf16, kind="ExternalInput")
    WaT = nc.dram_tensor("WaT", [OUT, OUT], bf16, kind="ExternalInput")
    ba_x = nc.dram_tensor("ba_x", [128, OUT], f32, kind="ExternalInput")
    WihT = nc.dram_tensor("WihT", [OUT, 3 * OUT], bf16, kind="ExternalInput")
    bih_x = nc.dram_tensor("bih_x", [128, 3 * OUT], f32, kind="ExternalInput")
    WhhT = nc.dram_tensor("WhhT", [OUT, 3 * OUT], bf16, kind="ExternalInput")
    bhh_x = nc.dram_tensor("bhh_x", [128, 3 * OUT], f32, kind="ExternalInput")
    meta = nc.dram_tensor("meta", [R * G * 128, 4], i32, kind="ExternalInput")
    mslot = nc.dram_tensor("mslot", [R * G * 128, 4], i16,
                           kind="ExternalInput")
    qidx = nc.dram_tensor("qidx", [R * 128, G * 8], i16, kind="ExternalInput")
    sidx = nc.dram_tensor("sidx", [R * 128, G * 8], i16, kind="ExternalInput")
    h_out = nc.dram_tensor("h_out", [SHP, OUT], f32, kind="ExternalOutput")

    scratch = []
    for t in range(nsteps):
        s = {
            "kv_loc": nc.dram_tensor(f"kv_loc{t}", [SHP, 2 * OUT], bf16),
            "kv_full": nc.dram_tensor(f"kv_full{t}", [KVN, 2 * OUT], bf16,
                                      addr_space="Shared"),
            "qr": nc.dram_tensor(f"qr{t}", [SHP, R * OUT], bf16),
            "agg": nc.dram_tensor(f"agg{t}", [AGGR, OUT], f32),
        }
        scratch.append(s)

    with tile.TileContext(nc) as tc:
        with tc.tile_pool(name="const", bufs=1) as cp:
            identf = cp.tile([128, 128], f32)
            make_identity(nc, identf[:])
            identb = cp.tile([128, 128], bf16)
            make_identity(nc, identb[:])
            iot = cp.tile([128, 4 * 64], i16)
            nc.gpsimd.iota(iot[:], pattern=[[0, 4], [1, 64]],
                           channel_multiplier=0)
            wkv_t = cp.tile([128, 2 * OUT], bf16)
            nc.sync.dma_start(out=wkv_t[:], in_=Wkv[:])
            bkv_t = cp.tile([128, 2 * OUT], f32)
            nc.sync.dma_start(out=bkv_t[:], in_=bkv_x[:])
            wqr_t = cp.tile([128, R * OUT], bf16)
            nc.sync.dma_start(out=wqr_t[:], in_=Wqr[:])
            bqr_t = cp.tile([128, R * OUT], f32)
            nc.sync.dma_start(out=bqr_t[:], in_=bqr_x[:])
            bdm_t = cp.tile([128, R * OUT], bf16)
            nc.sync.dma_start(out=bdm_t[:], in_=BDm[:])
            wa_t = cp.tile([128, OUT], bf16)
            nc.sync.dma_start(out=wa_t[:], in_=WaT[:])
            ba_t = cp.tile([128, OUT], f32)
            nc.sync.dma_start(out=ba_t[:], in_=ba_x[:])
            wih_t = cp.tile([128, 3 * OUT], bf16)
            nc.sync.dma_start(out=wih_t[:], in_=WihT[:])
            bih_t = cp.tile([128, 3 * OUT], f32)
            nc.sync.dma_start(out=bih_t[:], in_=bih_x[:])
            whh_t = cp.tile([128, 3 * OUT], bf16)
            nc.sync.dma_start(out=whh_t[:], in_=WhhT[:])
            bhh_t = cp.tile([128, 3 * OUT], f32)
            nc.sync.dma_start(out=bhh_t[:], in_=bhh_x[:])
            # persistent h state: f32 rows + bf16 transposed (feature-major)
            hstore = cp.tile([128, NTILE * 128], f32)
            hbT = cp.tile([128, NTILE * 128], bf16)
            zeros = cp.tile([128, 8 * OUT], f32)
            nc.vector.memset(zeros[:], 0.0)

            consts = dict(identf=identf, identb=identb, iot=iot,
                          wkv_t=wkv_t, bkv_t=bkv_t, wqr_t=wqr_t, bqr_t=bqr_t,
                          bdm_t=bdm_t, wa_t=wa_t, ba_t=ba_t, wih_t=wih_t,
                          bih_t=bih_t, whh_t=whh_t, bhh_t=bhh_t,
                          hstore=hstore, hbT=hbT, zeros=zeros)

            for t in range(nsteps):
                _emit_step(nc, tc, G, consts, scratch[t],
                           meta, mslot, qidx, sidx,
                           h_src=h_in if t == 0 else None,
                           h_dst=h_out if t == nsteps - 1 else None)

    nc.compile()
    return nc


def _emit_dense(nc, tc, c, s, h_src):
    """Load h (step 0 only), build hstore/hbT, compute kv (bf16)."""
    with tc.tile_pool(name="d1s", bufs=3) as sb, \
         tc.tile_pool(name="d1p", bufs=2, space="PSUM") as ps:
        for i in range(NTILE // ST):
            sl = slice(i * ST * 128, (i + 1) * ST * 128)
            csl = slice(i * ST * 128, (i + 1) * ST * 128)
            if h_src is not None:
                nc.sync.dma_start(
                    out=c["hstore"][:, csl].rearrange(
                        "p (t f) -> p t f", t=ST),
                    in_=h_src[sl, :].rearrange("(t p) f -> p t f", p=128))
                hb = sb.tile([128, ST * 128], bf16, tag="hb")
                nc.vector.tensor_copy(out=hb[:], in_=c["hstore"][:, csl])
                tp = ps.tile([128, ST * 128], bf16, space="PSUM", tag="tp")
                for k in range(ST):
                    nc.tensor.transpose(
                        out=tp[:, k * 128:(k + 1) * 128],
                        in_=hb[:, k * 128:(k + 1) * 128],
                        identity=c["identb"][:])
                nc.scalar.activation(out=c["hbT"][:, csl], in_=tp[:],
                                     func=AF.Copy)
            kvp = ps.tile([128, ST * 256], f32, space="PSUM", tag="kvp")
            for k in range(ST):
                nc.tensor.matmul(
                    out=kvp[:, k * 256:(k + 1) * 256],
                    lhsT=c["hbT"][:, i * ST * 128 + k * 128:
                                  i * ST * 128 + (k + 1) * 128],
                    rhs=c["wkv_t"][:], start=True, stop=True)
            kvt = sb.tile([128, ST * 256], bf16, tag="kvt")
            nc.vector.tensor_tensor(
                out=kvt[:].rearrange("p (t f) -> p t f", t=ST),
                in0=kvp[:].rearrange("p (t f) -> p t f", t=ST),
                in1=c["bkv_t"][:].unsqueeze(1).broadcast_to([128, ST, 256]),
                op=ALU.add)
            nc.sync.dma_start(
                out=s["kv_loc"][sl, :].rearrange("(t p) f -> p t f", p=128),
                in_=kvt[:].rearrange("p (t f) -> p t f", t=ST))


def _emit_qr_and_zero(nc, tc, c, s):
    """Compute qr (bf16, all 3 relations) and zero the agg table."""
    with tc.tile_pool(name="d2s", bufs=3) as sb, \
         tc.tile_pool(name="d2p", bufs=2, space="PSUM") as ps:
        for i in range(NTILE // ST):
            sl = slice(i * ST * 128, (i + 1) * ST * 128)
            qrt = sb.tile([128, ST * 384], bf16, tag="qrt")
            for k in range(ST):
                qrp = ps.tile([128, 384], f32, space="PSUM", tag=f"qrp{k}")
                nc.tensor.matmul(
                    out=qrp[:],
                    lhsT=c["hbT"][:, i * ST * 128 + k * 128:
                                  i * ST * 128 + (k + 1) * 128],
                    rhs=c["wqr_t"][:], start=True, stop=True)
                nc.vector.tensor_add(
                    out=qrt[:, k * 384:(k + 1) * 384],
                    in0=qrp[:], in1=c["bqr_t"][:])
            nc.sync.dma_start(
                out=s["qr"][sl, :].rearrange("(t p) f -> p t f", p=128),
                in_=qrt[:].rearrange("p (t f) -> p t f", t=ST))
        for i in range(AGGR // 1024):
            nc.sync.dma_start(
                out=s["agg"][i * 1024:(i + 1) * 1024, :].rearrange(
                    "(t p) f -> p t f", p=128),
                in_=c["zeros"][:].rearrange("p (t f) -> p t f", t=8))


def _emit_edge(nc, tc, G, c, s, meta, mslot, qidx, sidx):
    for r in range(DBG_RELS):
        gcap = min(G, DBG_GCAP)
        nbatch = (gcap + NB - 1) // NB
        with tc.tile_pool(name=f"ebs{r}", bufs=2) as bb, \
             tc.tile_pool(name=f"egs{r}", bufs=4) as sb, \
             tc.tile_pool(name=f"egp{r}", bufs=2, space="PSUM") as ps, \
             tc.tile_pool(name=f"egq{r}", bufs=1, space="PSUM") as ps1:
            for b in range(nbatch):
                g0 = b * NB
                gn = min(NB, gcap - g0)
                nidx = gn * 128
                mt = bb.tile([128, NB * 4], i32, tag="mt")
                nc.sync.dma_start(
                    out=mt[:, :gn * 4].rearrange("p (g k) -> p g k", g=gn),
                    in_=meta[(r * G + g0) * 128:(r * G + g0 + gn) * 128, :]
                    .rearrange("(g p) k -> p g k", p=128))
                mts = bb.tile([128, NB * 4], i16, tag="mts")
                nc.sync.dma_start(
                    out=mts[:, :gn * 4].rearrange("p (g k) -> p g k", g=gn),
                    in_=mslot[(r * G + g0) * 128:(r * G + g0 + gn) * 128, :]
                    .rearrange("(g p) k -> p g k", p=128))
                qi = bb.tile([128, NB * 8], i16, tag="qi")
                nc.sync.dma_start(
                    out=qi[:, :gn * 8],
                    in_=qidx[r * 128:(r + 1) * 128, g0 * 8:(g0 + gn) * 8])
                si = bb.tile([128, NB * 8], i16, tag="si")
                nc.sync.dma_start(
                    out=si[:, :gn * 8],
                    in_=sidx[r * 128:(r + 1) * 128, g0 * 8:(g0 + gn) * 8])
                qsl = bb.tile([128, NB * 128], bf16, tag="qsl")
                nc.gpsimd.dma_gather(
                    out_ap=qsl[:, :gn * 128].rearrange(
                        "p (g f) -> p g f", f=128),
                    in_ap=s["qr"][:, r * 128:(r + 1) * 128],
                    idxs_ap=qi[:, :gn * 8], num_idxs=nidx, num_idxs_reg=nidx,
                    elem_size=128, elem_step=R * OUT)
                # base-64 PE operands fault on this runner: shift the upper
                # half (chunks 2,3 slot rows) down to a base-0 tile
                qslB = bb.tile([64, NB * 128], bf16, tag="qslB")
                nc.sync.dma_start(out=qslB[:, :gn * 128],
                                  in_=qsl[64:128, :gn * 128])
                resA = bb.tile([128, NB * 64], f32, tag="resA")
                resB = bb.tile([128, NB * 64], f32, tag="resB")
                for gi_ in range(gn):
                    g = g0 + gi_
                    moff = gi_ * 4
                    kvg = sb.tile([128, 4 * 256], bf16, tag="kvg")
                    for c4 in range(4):
                        nc.gpsimd.indirect_dma_start(
                            out=kvg[:, c4 * 256:(c4 + 1) * 256],
                            out_offset=None, in_=s["kv_full"][:],
                            in_offset=bass.IndirectOffsetOnAxis(
                                ap=mt[:, moff + c4:moff + c4 + 1],
                                axis=0))
                    if DBG_ELVL < 2:
                        continue
                    # one-hot [128e, 4*64]: block c4 col j -> slotv == j
                    oh = sb.tile([128, 4 * 64], bf16, tag="oh")
                    nc.vector.tensor_tensor(
                        out=oh[:].rearrange("p (c j) -> p c j", c=4),
                        in0=mts[:, moff:moff + 4]
                        .unsqueeze(2).broadcast_to([128, 4, 64]),
                        in1=c["iot"][:].rearrange("p (c j) -> p c j", c=4),
                        op=ALU.is_equal)
                    if DBG_E2 < 1:
                        continue
                    # transpose each 64-col block -> all-base-0 [64, 512]
                    ohtp = ps.tile([64, 4 * 128], bf16, space="PSUM",
                                   tag="ohtp")
                    for c4 in range(4):
                        nc.tensor.transpose(
                            out=ohtp[:, c4 * 128:(c4 + 1) * 128],
                            in_=oh[:, c4 * 64:(c4 + 1) * 64],
                            identity=c["identb"][:])
                    if DBG_E2 < 2:
                        continue
                    ohts = sb.tile([64, 4 * 128], bf16, tag="ohts")
                    nc.scalar.activation(out=ohts[:], in_=ohtp[:],
                                         func=AF.Copy)
                    if DBG_E2 < 3:
                        continue
                    # qep[e, c4*128+f] = q row of edge's slot
                    qep = ps.tile([128, 4 * 128], f32, space="PSUM",
                                  tag="qep")
                    for c4 in range(4):
                        rq = qsl if c4 < 2 else qslB
                        nc.tensor.matmul(
                            out=qep[:, c4 * 128:(c4 + 1) * 128],
                            lhsT=ohts[:, c4 * 128:(c4 + 1) * 128],
                            rhs=rq[0:64, gi_ * 128:(gi_ + 1) * 128],
                            start=True, stop=True)
                    if DBG_ELVL < 3:
                        continue
                    # SDDMM score + exp (qep copied to bf16 on ACT so the
                    # multiply runs at 16-bit DVE rate)
                    qeb = sb.tile([128, 512], bf16, tag="qeb")
                    nc.scalar.activation(out=qeb[:], in_=qep[:], func=AF.Copy)
                    qk = sb.tile([128, 512], bf16, tag="qk")
                    nc.vector.tensor_tensor(
                        out=qk[:].rearrange("p (c f) -> p c f", c=4),
                        in0=kvg[:].rearrange("p (ch two f) -> p ch two f",
                                             ch=4, two=2)[:, :, 0, :],
                        in1=qeb[:].rearrange("p (c f) -> p c f", c=4),
                        op=ALU.mult)
                    score = sb.tile([128, 32], bf16, tag="score")
                    with nc.allow_low_precision(reason="bf16 attn scores, "
                                                "tolerance 2e-2"):
                        nc.vector.tensor_reduce(
                            out=score[:],
                            in_=qk[:].rearrange("p (s d) -> p s d", d=DK),
                            axis=AX.X, op=ALU.add)
                    ex = sb.tile([128, 32], bf16, tag="ex")
                    nc.scalar.activation(out=ex[:], in_=score[:], func=AF.Exp)
                    # weighted V
                    vw = sb.tile([128, 512], bf16, tag="vw")
                    nc.vector.tensor_tensor(
                        out=vw[:].rearrange("p (ch hh d) -> p ch hh d",
                                            ch=4, hh=H),
                        in0=kvg[:].rearrange("p (ch two hh d) -> p ch two hh d",
                                             ch=4, two=2, hh=H)[:, :, 1, :, :],
                        in1=ex[:].rearrange("p (ch hh) -> p ch hh", ch=4)
                        .unsqueeze(3).broadcast_to([128, 4, H, DK]),
                        op=ALU.mult)
                    if DBG_ELVL < 4:
                        continue
                    # segment sums (one-hot matmuls)
                    up = ps1.tile([128, 128], f32, space="PSUM", tag="up")
                    sfp = ps1.tile([8, 128], f32, space="PSUM", tag="sfp")
                    for c4 in range(4):
                        ohc = oh[:, c4 * 64 + 32 * (c4 & 1):
                                 c4 * 64 + 32 * (c4 & 1) + 32]
                        nc.tensor.matmul(
                            out=up[:, c4 * 32:(c4 + 1) * 32],
                            lhsT=vw[:, c4 * 128:(c4 + 1) * 128],
                            rhs=ohc, start=True, stop=True)
                        nc.tensor.matmul(
                            out=sfp[:, c4 * 32:(c4 + 1) * 32],
                            lhsT=ex[:, c4 * 8:(c4 + 1) * 8],
                            rhs=ohc, start=True, stop=True)
                    if DBG_ELVL < 5:
                        continue
                    usb = sb.tile([128, 128], bf16, tag="usb")
                    nc.scalar.activation(out=usb[:], in_=up[:], func=AF.Copy)
                    sfc = sb.tile([8, 128], f32, tag="sfc")
                    nc.scalar.activation(out=sfc[:], in_=sfp[:], func=AF.Copy)
                    ampt = ps1.tile([128, 136], f32, space="PSUM", tag="ampt")
                    nc.tensor.transpose(out=ampt[:, 128:136], in_=sfc[:],
                                        identity=c["identf"][0:8, 0:8])
                    rs = sb.tile([128, 8], f32, tag="rs")
                    nc.vector.tensor_scalar_add(rs[:], ampt[:, 128:136], 1e-30)
                    nc.vector.reciprocal(out=rs[:], in_=rs[:])
                    amp = ampt[:, 0:128]
                    nc.tensor.matmul(
                        out=amp, lhsT=usb[:],
                        rhs=c["bdm_t"][:, r * OUT:(r + 1) * OUT],
                        start=True, stop=True)
                    # res halves (normalized)
                    nc.vector.tensor_tensor(
                        out=resA[:, gi_ * 64:(gi_ + 1) * 64]
                        .rearrange("p (hh d) -> p hh d", hh=4),
                        in0=ampt[:, 0:64].rearrange("p (hh d) -> p hh d", hh=4),
                        in1=rs[:, 0:4].unsqueeze(2)
                        .broadcast_to([128, 4, DK]),
                        op=ALU.mult)
                    nc.vector.tensor_tensor(
                        out=resB[:, gi_ * 64:(gi_ + 1) * 64]
                        .rearrange("p (hh d) -> p hh d", hh=4),
                        in0=ampt[:, 64:128].rearrange("p (hh d) -> p hh d",
                                                     hh=4),
                        in1=rs[:, 4:8].unsqueeze(2)
                        .broadcast_to([128, 4, DK]),
                        op=ALU.mult)
                if DBG_ELVL < 6:
                    continue
                nc.gpsimd.dma_scatter_add(
                    out_ap=s["agg"][:, 0:64],
                    in_ap=resA[:, :gn * 64].rearrange("p (g f) -> p g f",
                                                      f=64),
                    idxs_ap=si[:, :gn * 8], num_idxs=nidx, num_idxs_reg=nidx,
                    elem_size=64, elem_step=OUT)
                nc.gpsimd.dma_scatter_add(
                    out_ap=s["agg"][:, 64:128],
                    in_ap=resB[:, :gn * 64].rearrange("p (g f) -> p g f",
                                                      f=64),
                    idxs_ap=si[:, :gn * 8], num_idxs=nidx, num_idxs_reg=nidx,
                    elem_size=64, elem_step=OUT)


def _emit_gru(nc, tc, c, s, h_dst):
    with tc.tile_pool(name="gs", bufs=3) as sb, \
         tc.tile_pool(name="gp", bufs=1, space="PSUM") as ps:
        for i in range(NTILE // ST):
            sl = slice(i * ST * 128, (i + 1) * ST * 128)
            csl = slice(i * ST * 128, (i + 1) * ST * 128)
            agt = sb.tile([128, ST * 128], f32, tag="agt")
            nc.sync.dma_start(
                out=agt[:].rearrange("p (t f) -> p t f", t=ST),
                in_=s["agg"][sl, :].rearrange("(t p) f -> p t f", p=128))
            tgb = sb.tile([128, ST * 128], bf16, tag="tgb")
            nc.scalar.activation(out=tgb[:], in_=agt[:], func=AF.Gelu)
            ttp = ps.tile([128, ST * 128], bf16, space="PSUM", tag="ttp")
            for k in range(ST):
                nc.tensor.transpose(out=ttp[:, k * 128:(k + 1) * 128],
                                    in_=tgb[:, k * 128:(k + 1) * 128],
                                    identity=c["identb"][:])
            tT = sb.tile([128, ST * 128], bf16, tag="tT")
            nc.scalar.activation(out=tT[:], in_=ttp[:], func=AF.Copy)
            xp = ps.tile([128, ST * 128], f32, space="PSUM", tag="xp")
            for k in range(ST):
                nc.tensor.matmul(out=xp[:, k * 128:(k + 1) * 128],
                                 lhsT=tT[:, k * 128:(k + 1) * 128],
                                 rhs=c["wa_t"][:], start=True, stop=True)
            xtb = sb.tile([128, ST * 128], bf16, tag="xtb")
            nc.vector.tensor_tensor(
                out=xtb[:].rearrange("p (t f) -> p t f", t=ST),
                in0=xp[:].rearrange("p (t f) -> p t f", t=ST),
                in1=c["ba_t"][:].unsqueeze(1).broadcast_to([128, ST, 128]),
                op=ALU.add)
            xtp = ps.tile([128, ST * 128], bf16, space="PSUM", tag="xtp")
            for k in range(ST):
                nc.tensor.transpose(out=xtp[:, k * 128:(k + 1) * 128],
                                    in_=xtb[:, k * 128:(k + 1) * 128],
                                    identity=c["identb"][:])
            xT = sb.tile([128, ST * 128], bf16, tag="xT")
            nc.scalar.activation(out=xT[:], in_=xtp[:], func=AF.Copy)
            for k in range(ST):
                ksl = slice(i * ST * 128 + k * 128, i * ST * 128 + (k + 1) * 128)
                gip = ps.tile([128, 3 * OUT], f32, space="PSUM", tag="gip")
                nc.tensor.matmul(out=gip[:], lhsT=xT[:, k * 128:(k + 1) * 128],
                                 rhs=c["wih_t"][:], start=True, stop=True)
                ghp = ps.tile([128, 3 * OUT], f32, space="PSUM", tag="ghp")
                nc.tensor.matmul(out=ghp[:], lhsT=c["hbT"][:, ksl],
                                 rhs=c["whh_t"][:], start=True, stop=True)
                gi_t = sb.tile([128, 3 * OUT], f32, tag=f"gi{k}")
                nc.vector.tensor_add(out=gi_t[:], in0=gip[:], in1=c["bih_t"][:])
                gh_t = sb.tile([128, 3 * OUT], f32, tag=f"gh{k}")
                nc.vector.tensor_add(out=gh_t[:], in0=ghp[:], in1=c["bhh_t"][:])
                rg = sb.tile([128, OUT], f32, tag=f"rg{k}")
                nc.vector.tensor_add(out=rg[:], in0=gi_t[:, 0:OUT],
                                     in1=gh_t[:, 0:OUT])
                nc.scalar.activation(out=rg[:], in_=rg[:], func=AF.Sigmoid)
                zg = sb.tile([128, OUT], f32, tag=f"zg{k}")
                nc.vector.tensor_add(out=zg[:], in0=gi_t[:, OUT:2 * OUT],
                                     in1=gh_t[:, OUT:2 * OUT])
                nc.scalar.activation(out=zg[:], in_=zg[:], func=AF.Sigmoid)
                ng = sb.tile([128, OUT], f32, tag=f"ng{k}")
                nc.vector.tensor_mul(out=ng[:], in0=rg[:],
                                     in1=gh_t[:, 2 * OUT:3 * OUT])
                nc.vector.tensor_add(out=ng[:], in0=ng[:],
                                     in1=gi_t[:, 2 * OUT:3 * OUT])
                nc.scalar.activation(out=ng[:], in_=ng[:], func=AF.Tanh)
                # h' = n + z*(h - n)
                hm = sb.tile([128, OUT], f32, tag=f"hm{k}")
                nc.vector.tensor_tensor(out=hm[:], in0=c["hstore"][:, ksl],
                                        in1=ng[:], op=ALU.subtract)
                nc.vector.tensor_mul(out=hm[:], in0=hm[:], in1=zg[:])
                nc.vector.tensor_add(out=hm[:], in0=hm[:], in1=ng[:])
                if h_dst is not None:
                    nc.sync.dma_start(
                        out=h_dst[i * ST * 128 + k * 128:
                                  i * ST * 128 + (k + 1) * 128, :],
                        in_=hm[:])
                else:
                    # next step state: f32 rows + transposed bf16
                    nc.vector.tensor_copy(out=c["hstore"][:, ksl], in_=hm[:])
                    hmb = sb.tile([128, OUT], bf16, tag=f"hmb{k}")
                    nc.vector.tensor_copy(out=hmb[:], in_=hm[:])
                    htp = ps.tile([128, 128], bf16, space="PSUM", tag="htp")
                    nc.tensor.transpose(out=htp[:], in_=hmb[:],
                                        identity=c["identb"][:])
                    nc.scalar.activation(out=c["hbT"][:, ksl],
                                         in_=htp[:], func=AF.Copy)


def _emit_step(nc, tc, G, c, s, meta, mslot, qidx, sidx, h_src, h_dst):
    _emit_dense(nc, tc, c, s, h_src)
    nc.gpsimd.collective_compute(
        "AllGather", ALU.bypass,
        replica_groups=[list(range(NCORES))],
        ins=[s["kv_loc"][:].opt()],
        outs=[s["kv_full"][:].opt()],
    )
    _emit_qr_and_zero(nc, tc, c, s)
    _emit_edge(nc, tc, G, c, s, meta, mslot, qidx, sidx)
    _emit_gru(nc, tc, c, s, h_dst)


# --------------------------------------------------------------------------
# Entry point
# --------------------------------------------------------------------------

_CACHE = {}


def _prepare(inputs):
    h = np.asarray(inputs["h"]).astype(np.float32)
    G, per_core = build_graph_data(inputs["src"], inputs["dst"])
    w = fold_weights(inputs)
    in_maps = []
    for cidx in range(NCORES):
        hs = np.zeros((SHP, IN), np.float32)
        hs[:SH] = h[cidx * SH:(cidx + 1) * SH]
        m = {"h_in": hs, "meta": per_core[cidx]["meta"],
             "mslot": per_core[cidx]["mslot"],
             "qidx": per_core[cidx]["qidx"], "sidx": per_core[cidx]["sidx"]}
        m.update(w)
        in_maps.append(m)
    return G, in_maps


def kernel(**inputs) -> np.ndarray:
    G, in_maps = _prepare(inputs)
    if G not in _CACHE:
        _CACHE[G] = build_program(G)
    nc = _CACHE[G]
    from concourse.bass_utils import run_bass_kernel_spmd
    r = run_bass_kernel_spmd(nc, in_maps, list(range(NCORES)))
    out = np.empty((N, OUT), np.float32)
    for cidx in range(NCORES):
        out[cidx * SH:(cidx + 1) * SH] = r.results[cidx]["h_out"][:SH]
    return out
